# revision 1
# baseline (speedup 1.0000x reference)
"""GNN message-passing kernel for Trainium2 (8 NeuronCores).

Reference computation:
    out[b,i,f] = X[b,0,i,i,f] + sum_{k=1..3} sum_j A[b,i,j] * X[b,k,i,j,f]

Sharding: 8 cores = (batch b in 0..3) x (i-half h in 0..1); each core owns
a (b, 128-row i-slab) of the output. Hop 0 only contributes its diagonal,
so only X[b,1:4] (3/4 of X) plus the hop-0 diagonal rows are ever sent to
the device: ~25 MB per core.

Per-core device kernel:
  - X slabs are DMA'd in their NATURAL layout: partition = i (128 rows),
    free = (j, f) flattened, in variable j-chunks (small ones first so
    compute starts ~7us earlier). Each partition's data is one fully
    contiguous run -> near-peak HBM bandwidth (~414 GB/s measured vs
    ~193 GB/s for a transposed layout).
  - out[i,f] = sum_j A[i,j] * (sum_k X[k])[i,j,f]: the hop sum runs on
    the VectorEngine (two adds) for most chunks, and on the TensorEngine
    (identity-stationary matmuls accumulating into PSUM, after a HAM
    warm-up burst) for two early chunks to offload DVE. Then a
    broadcast-AP multiply (A[i,j] broadcast over f via a 0-step AP dim)
    and a strided tensor_reduce over j per chunk; the hop-0 diagonal is
    added into the running accumulator.

Measured on 8 axon-tunneled trn2 cores: ~107.3 us HW exec, rel err
~2e-7 (DMA ~61 us at ~414 GB/s, DVE ~73 us, overlapped; DVE's 4 passes
over the data are the algorithmic floor for fp32 on this ISA).
Variants tried and rejected: j-on-partition matmul formulation (162 us,
256B DMA descriptors dominate), SWDGE accumulate-DMA hop sum (device
crash), GpSimd assist (SBUF port contention slows DVE), full TensorE
identity-matmul hop-sum (fp32 dual-pass makes PE the bottleneck).
"""

import sys

if "/opt/trn_rl_repo" not in sys.path:
    sys.path.insert(0, "/opt/trn_rl_repo")

import numpy as np

import concourse.bacc as bacc
import concourse.bass as bass
import concourse.mybir as mybir
from concourse.bass_utils import run_bass_kernel_spmd
from concourse.tile import TileContext

BATCH, KP1, N, F = 4, 4, 256, 64
NH = N // 2          # 128 rows of output per core (partition dim)
# j-chunk sizes: small chunks first so DVE starts ~7us earlier.
# Chunks 1,2 get their hop-sum done on the TensorEngine (identity-matmul
# accumulate into PSUM) -- back-to-back so the HAM cold-start is paid once.
CJS = [32, 32, 32, 64, 64, 32]
PE_CHUNKS = {1, 2}
FP32 = mybir.dt.float32

_CACHE = {}


def _build_nc():
    if "nc" in _CACHE:
        return _CACHE["nc"]
    nc = bacc.Bacc("TRN2", target_bir_lowering=False, debug=False, num_devices=8)
    xk = nc.dram_tensor("xk", [3, NH, N, F], FP32, kind="ExternalInput").ap()
    a = nc.dram_tensor("a", [NH, N], FP32, kind="ExternalInput").ap()
    d = nc.dram_tensor("d", [NH, F], FP32, kind="ExternalInput").ap()
    eye = nc.dram_tensor("eye", [128, 128], FP32, kind="ExternalInput").ap()
    out = nc.dram_tensor("out", [NH, F], FP32, kind="ExternalOutput").ap()

    with TileContext(nc) as tc:
        with (
            tc.tile_pool(name="const", bufs=1) as cpool,
            tc.tile_pool(name="xs", bufs=3) as xpool,
            tc.tile_pool(name="pr", bufs=2) as prpool,
            tc.tile_pool(name="sm", bufs=2) as smpool,
            tc.tile_pool(name="ac", bufs=1) as acpool,
            tc.tile_pool(name="ps", bufs=2, space="PSUM") as pspool,
        ):
            a_sb = cpool.tile([128, N], FP32)
            nc.sync.dma_start(out=a_sb[:, :], in_=a[:, :])
            d_sb = cpool.tile([128, F], FP32)
            nc.sync.dma_start(out=d_sb[:, :], in_=d[:, :])
            eye_sb = cpool.tile([128, 128], FP32)
            nc.sync.dma_start(out=eye_sb[:, :], in_=eye[:, :])

            acc = acpool.tile([128, F], FP32)

            # PE warm-up: ~16 dummy matmuls trip the HAM activity window
            # (~3.4us) so the real chunk-1/2 matmuls run at 2.4 GHz, not
            # the 1.2 GHz cold clock. Output is never read.
            warm = pspool.tile([128, 512], FP32, name="ps", tag="ps")
            for _ in range(24):
                nc.tensor.matmul(
                    warm[:, 0:N],
                    eye_sb[:, :],
                    a_sb[:, :],
                    start=True,
                    stop=True,
                )

            j0 = 0
            for c, CJ in enumerate(CJS):
                xts = []
                for k in range(3):
                    xt = xpool.tile(
                        [128, CJ * F], FP32, name=f"xt{k}", tag=f"xt{k}"
                    )
                    src = bass.AP(
                        xk.tensor,
                        k * NH * N * F + j0 * F,
                        [[N * F, 128], [1, CJ * F]],
                    )
                    nc.sync.dma_start(out=xt[:, :], in_=src)
                    xts.append(xt)
                if c in PE_CHUNKS:
                    # hop sum on the TensorEngine: identity-stationary
                    # matmuls accumulate x1+x2+x3 into PSUM; PE reads SBUF
                    # through its own ports, so DVE is not slowed
                    ps = pspool.tile([128, CJ * F], FP32, name="ps", tag="ps")
                    for s in range((CJ * F) // 512):
                        sl = slice(s * 512, (s + 1) * 512)
                        for k in range(3):
                            nc.tensor.matmul(
                                ps[:, sl],
                                eye_sb[:, :],
                                xts[k][:, sl],
                                start=(k == 0),
                                stop=(k == 2),
                            )
                    xs = ps
                else:
                    # hop sum on DVE (in place)
                    nc.vector.tensor_add(xts[1][:, :], xts[1][:, :], xts[2][:, :])
                    nc.vector.tensor_add(xts[0][:, :], xts[0][:, :], xts[1][:, :])
                    xs = xts[0]
                xs_step = xs.ap[0][0]

                # prod[i, j*F+f] = xs[i, j*F+f] * a_sb[i, c*CJ+j]
                prod = prpool.tile([128, CJ * F], FP32, name="prod", tag="prod")
                pr_step = prod.ap[0][0]
                a_step = a_sb.ap[0][0]
                in0 = bass.AP(xs.tensor, 0, [[xs_step, 128], [F, CJ], [1, F]])
                in1 = bass.AP(
                    a_sb.tensor, j0, [[a_step, 128], [1, CJ], [0, F]]
                )
                j0 += CJ
                po = bass.AP(prod.tensor, 0, [[pr_step, 128], [F, CJ], [1, F]])
                nc.vector.tensor_mul(po, in0, in1)

                # partial[i, f] = sum_j prod[i, j*F+f]  (reduce innermost=j)
                partial = smpool.tile([128, F], FP32, name="partial", tag="partial")
                pin = bass.AP(prod.tensor, 0, [[pr_step, 128], [1, F], [F, CJ]])
                nc.vector.reduce_sum(
                    partial[:, :], pin, axis=mybir.AxisListType.X
                )

                if c == 0:
                    nc.vector.tensor_add(acc[:, :], d_sb[:, :], partial[:, :])
                else:
                    nc.vector.tensor_add(acc[:, :], acc[:, :], partial[:, :])

            nc.sync.dma_start(out=out[:, :], in_=acc[:, :])

    nc.compile()
    _CACHE["nc"] = nc
    return nc


def _make_in_maps(A, X):
    idx = np.arange(NH)
    in_maps = []
    for c in range(8):
        b, h = c // 2, c % 2
        lo = h * NH
        xk = np.ascontiguousarray(X[b, 1:4, lo : lo + NH])
        av = np.ascontiguousarray(A[b, lo : lo + NH, :])
        dv = np.ascontiguousarray(X[b, 0, lo + idx, lo + idx, :])
        in_maps.append(
            {"xk": xk, "a": av, "d": dv, "eye": np.eye(128, dtype=np.float32)}
        )
    return in_maps


def run(A, X, trace=False, **kw):
    nc = _build_nc()
    in_maps = _make_in_maps(A, X)
    res = run_bass_kernel_spmd(
        nc, in_maps, core_ids=list(range(8)), trace=trace, **kw
    )
    out = np.empty((BATCH, N, F), dtype=np.float32)
    for c in range(8):
        b, h = c // 2, c % 2
        out[b, h * NH : (h + 1) * NH] = res.results[c]["out"]
    return out, res


def kernel(A, X):
    A = np.asarray(A, dtype=np.float32)
    X = np.asarray(X, dtype=np.float32)
    out, _ = run(A, X, trace=False)
    return out



# revision 3
# speedup vs baseline: 1.7961x; 1.7961x over previous
"""GNN message-passing kernel for Trainium2 (8 NeuronCores).

Reference computation:
    out[b,i,f] = X[b,0,i,i,f] + sum_{k=1..3} sum_j A[b,i,j] * X[b,k,i,j,f]

Sharding: 8 cores = (batch b in 0..3) x (i-half h in 0..1); each core owns
a (b, 128-row i-slab) of the output.

Strategy (v2, matmul formulation in fp16):
  - Host pre-transposes X[b,1:4,ilab] to X_t[j, i, k, f] fp16 and A to
    A_t[j, i] fp16. fp16 halves HBM traffic vs fp32 (12.6 MB/core) and the
    j-on-partition layout keeps every DMA partition run contiguous
    (CI*384 bytes), so DMA stays at the ~410 GB/s ceiling.
  - For each output row i, the TensorEngine does the entire reduction:
    6 matmuls (2 j-halves x 3 hops) with lhsT = A_t[:, i] (128x1, M=1) and
    rhs = X_t[jhalf][:, i, k, :] (128x64) accumulate sum_{k,j} A[i,j]*X[k,i,j,f]
    directly into a per-i (1,64) PSUM slot. PSUM absorbs both the j-
    contraction and the hop sum; the VectorEngine does almost nothing.
  - M=1 output rows can only land on 32-aligned PSUM partitions
    (tile_position), so i -> (bank=i//32, row=32*((i%32)//8), slot=i%8):
    32 i's per 2KB PSUM bank, 4 banks total. Each bank is pre-zeroed by
    4 "zero matmuls" (lhsT=0) that set every has_written bit, so all real
    matmuls run as pure accumulates (start=False) and never race the
    per-bank has_written clear.
  - Per bank: ScalarE evacuates PSUM->SBUF (128,512), a small SBUF->SBUF
    gather DMA reorders the 32 rows into natural i order; finally DVE adds
    the hop-0 diagonal (fp32, exact) and the result is DMA'd out.
  - ~48 warmup matmuls + the zero-matmuls trip the HAM activity window
    during the initial DMA so real matmuls run at 2.4 GHz.
"""

import sys

if "/opt/trn_rl_repo" not in sys.path:
    sys.path.insert(0, "/opt/trn_rl_repo")

import numpy as np

import concourse.bacc as bacc
import concourse.bass as bass
import concourse.mybir as mybir
from concourse.bass_utils import run_bass_kernel_spmd
from concourse.tile import TileContext

BATCH, KP1, N, F = 4, 4, 256, 64
NH = N // 2          # 128 rows of output per core
KH = 3               # hops 1..3
CW = KH * F          # 192 fp16 values per (j, i) in X_t
# i-chunk sizes for the X DMAs: big first (BW), tapering tail so the last
# chunk's matmuls + evac + gather happen right after the DMA stream ends.
CIS = [32, 32, 24, 16, 12, 8, 4]
NWARM = 48
FP32 = mybir.dt.float32
FP16 = mybir.dt.float16

_CACHE = {}


def _build_nc():
    if "nc" in _CACHE:
        return _CACHE["nc"]
    nc = bacc.Bacc("TRN2", target_bir_lowering=False, debug=False, num_devices=8)
    xt = nc.dram_tensor("xt", [N, NH, KH, F], FP16, kind="ExternalInput").ap()
    at = nc.dram_tensor("at", [2, NH, NH], FP16, kind="ExternalInput").ap()
    d = nc.dram_tensor("d", [NH, F], FP32, kind="ExternalInput").ap()
    z = nc.dram_tensor("z", [1, 128], FP16, kind="ExternalInput").ap()
    out = nc.dram_tensor("out", [NH, F], FP32, kind="ExternalOutput").ap()

    starts = []
    s = 0
    for ci in CIS:
        starts.append(s)
        s += ci
    assert s == NH

    with TileContext(nc) as tc:
        with (
            tc.tile_pool(name="const", bufs=1) as cpool,
            tc.tile_pool(name="xs", bufs=1) as xpool,
            tc.tile_pool(name="ps", bufs=1, space="PSUM") as pspool,
        ):
            at_sb = []
            for h in range(2):
                t = cpool.tile([128, NH], FP16, name=f"at{h}", tag=f"at{h}")
                nc.sync.dma_start(
                    out=t[:, :],
                    in_=bass.AP(at.tensor, h * NH * NH, [[NH, 128], [1, NH]]),
                )
                at_sb.append(t)
            z_sb = cpool.tile([1, 128], FP16, name="z", tag="z")
            nc.sync.dma_start(out=z_sb[:, :], in_=z[:, :])
            d_sb = cpool.tile([128, F], FP32, name="d", tag="d")
            nc.sync.dma_start(out=d_sb[:, :], in_=d[:, :])

            # X chunk DMAs, all issued up front so the HWDGE ring never
            # stalls behind a semaphore wait from the gather DMAs.
            xts = {}
            for c, (s0, ci) in enumerate(zip(starts, CIS)):
                for h in range(2):
                    t = xpool.tile(
                        [128, ci * CW], FP16, name=f"x{h}_{c}", tag=f"x{h}_{c}"
                    )
                    src = bass.AP(
                        xt.tensor,
                        (h * 128) * (NH * CW) + s0 * CW,
                        [[NH * CW, 128], [1, ci * CW]],
                    )
                    nc.sync.dma_start(out=t[:, :], in_=src)
                    xts[(h, c)] = t

            ps = [
                pspool.tile([128, 512], FP32, name=f"ps{b}", tag=f"ps{b}")
                for b in range(4)
            ]
            warm = pspool.tile([128, 512], FP32, name="warm", tag="warm")
            E = cpool.tile([128, 2048], FP32, name="E", tag="E")
            G = cpool.tile([128, F], FP32, name="G", tag="G")
            out_sb = cpool.tile([128, F], FP32, name="out_sb", tag="out_sb")

            # HAM warmup: PE busy >=3.4us during the initial DMA stream.
            for w in range(NWARM):
                nc.tensor.matmul(
                    warm[:, 0:128], at_sb[0][:, :], at_sb[0][:, :],
                    start=True, stop=True,
                )

            e_step = E.ap[0][0]
            g_step = G.ap[0][0]

            c = 0  # current DMA chunk index
            for i in range(NH):
                while i >= starts[c] + CIS[c]:
                    c += 1
                ir = i - starts[c]
                b, q = i // 32, i % 32
                m, sl = q // 8, q % 8
                if q == 0:
                    # Zero bank b and set all its has_written bits: 4
                    # zero-weight matmuls; only the first clears the bank.
                    for v in range(4):
                        nc.tensor.matmul(
                            ps[b][:, v * 128:(v + 1) * 128],
                            z_sb[0:1, :],
                            at_sb[0][0:1, 0:128],
                            start=(v == 0),
                            stop=False,
                            skip_group_check=True,
                        )
                out_ap = ps[b][32 * m:32 * m + 1, sl * F:(sl + 1) * F]
                for h in range(2):
                    lhsT = at_sb[h][:, i:i + 1]
                    for k in range(KH):
                        nc.tensor.matmul(
                            out_ap,
                            lhsT,
                            xts[(h, c)][:, (ir * KH + k) * F:(ir * KH + k + 1) * F],
                            start=False,
                            stop=(q == 31 and h == 1 and k == KH - 1),
                            skip_group_check=True,
                            tile_position=(0, 32 * m),
                        )
                if q == 31:
                    # Evacuate bank b (ScalarE reads PSUM fast) and gather
                    # its 32 rows {0,32,64,96}x8 slots into natural i order.
                    nc.scalar.copy(E[:, b * 512:(b + 1) * 512], ps[b][:, :])
                    src = bass.AP(
                        E.tensor,
                        E.offset + b * 512,
                        [[32 * e_step, 4], [F, 8], [1, F]],
                    )
                    dst = bass.AP(
                        G.tensor,
                        G.offset + b * 32 * g_step,
                        [[g_step, 32], [1, F]],
                    )
                    nc.sync.dma_start(out=dst, in_=src)

            nc.vector.tensor_add(out_sb[:, :], G[:, :], d_sb[:, :])
            nc.sync.dma_start(out=out[:, :], in_=out_sb[:, :])

    nc.compile()
    _CACHE["nc"] = nc
    return nc


def _make_in_maps(A, X):
    idx = np.arange(NH)
    zz = np.zeros((1, 128), dtype=np.float16)
    Xh = X[:, 1:4].astype(np.float16)  # (4, 3, 256, 256, 64)
    in_maps = []
    for c in range(8):
        b, h = c // 2, c % 2
        lo = h * NH
        # X_t[j, i, k, f] = X[b, k+1, lo+i, j, f]
        xtv = np.ascontiguousarray(Xh[b, :, lo:lo + NH].transpose(2, 1, 0, 3))
        atv = np.ascontiguousarray(
            A[b, lo:lo + NH, :].T.astype(np.float16)
        ).reshape(2, NH, NH)
        dv = np.ascontiguousarray(X[b, 0, lo + idx, lo + idx, :]).astype(
            np.float32
        )
        in_maps.append({"xt": xtv, "at": atv, "d": dv, "z": zz})
    return in_maps


def run(A, X, trace=False, **kw):
    nc = _build_nc()
    in_maps = _make_in_maps(A, X)
    res = run_bass_kernel_spmd(
        nc, in_maps, core_ids=list(range(8)), trace=trace, **kw
    )
    out = np.empty((BATCH, N, F), dtype=np.float32)
    for c in range(8):
        b, h = c // 2, c % 2
        out[b, h * NH:(h + 1) * NH] = res.results[c]["out"]
    return out, res


def kernel(A, X):
    A = np.asarray(A, dtype=np.float32)
    X = np.asarray(X, dtype=np.float32)
    out, _ = run(A, X, trace=False)
    return out


# revision 5
# speedup vs baseline: 1.8913x; 1.0530x over previous
"""GNN message-passing kernel for Trainium2 (8 NeuronCores).

Reference computation:
    out[b,i,f] = X[b,0,i,i,f] + sum_{k=1..3} sum_j A[b,i,j] * X[b,k,i,j,f]

Sharding: 8 cores = (batch b in 0..3) x (i-half h in 0..1); each core owns
a (b, 128-row i-slab) of the output.

Strategy (v3, fp16 matmul formulation with 4-strip PE concurrency):
  - Host pre-transposes X[b,1:4,islab] to X_t[j, i, k, f] fp16 and A to
    A_t[j, i] fp16. fp16 halves HBM traffic vs fp32 (12.6 MB/core) and the
    j-on-partition layout keeps every DMA partition run contiguous, so the
    X stream runs at the ~420 GB/s DMA ceiling.
  - For each output row i the TensorEngine does the entire reduction:
    6 matmuls (2 j-halves x 3 hops) with lhsT = A_t[:, i] (128x1, M=1) and
    rhs = X_t[jhalf][:, i, k, :] (128x64) accumulate sum_{k,j} A[i,j]*X[k,i,j,f]
    into a per-i (1,64) PSUM slot. PSUM absorbs the j-contraction AND the
    hop sum; VectorE does nothing at all.
  - M=1 outputs land on 32-aligned PSUM partitions (tile_position), so
    slot(i): bank=i//32, strip row 32*(i%4), col 64*((i%32)//4). Matmuls
    are issued round-robin across the 4 col strips at single-matmul
    granularity, so 4 streams execute concurrently in the PE array
    (in-order starts, disjoint col groups) instead of serializing on one
    strip's 64-cycle stream.
  - Each bank is pre-seeded by one K=4 matmul (one-hot lhsT) that writes
    the hop-0 diagonal d[i,f] into every i's slot and sets the bank's
    has_written bits, so all real matmuls are pure accumulates
    (start=False) and the diagonal add costs nothing.
  - Per bank: ScalarE evacuates PSUM->SBUF (128,512) and a single
    strided out-DMA writes the 32 rows straight to HBM in natural i
    order. No gather, no vector work, minimal tail.
  - 48 warmup matmuls trip the HAM activity window during the initial
    DMA so real matmuls run at 2.4 GHz.

Measured v2 (no strip interleave, zero-seed + gather + add tail):
62.5 us, rel err 2.8e-4. DMA 7.2->40 us at ~420 GB/s; PE stream-bound at
~29 ns/matmul finishing 54.6 us; tail 8 us.
"""

import sys

if "/opt/trn_rl_repo" not in sys.path:
    sys.path.insert(0, "/opt/trn_rl_repo")

import numpy as np

import concourse.bacc as bacc
import concourse.bass as bass
import concourse.mybir as mybir
from concourse.bass_utils import run_bass_kernel_spmd
from concourse.tile import TileContext

BATCH, KP1, N, F = 4, 4, 256, 64
NH = N // 2          # 128 rows of output per core
KH = 3               # hops 1..3
CW = KH * F          # 192 fp16 values per (j, i) in X_t
# i-chunk sizes for the X DMAs: big first (BW), tapering tail so the last
# chunk's matmuls + evac + out-DMA happen right after the DMA stream ends.
CIS = [32, 32, 32, 16, 8, 8]
NWARM = 48
FP32 = mybir.dt.float32
FP16 = mybir.dt.float16

_CACHE = {}


def _build_nc():
    if "nc" in _CACHE:
        return _CACHE["nc"]
    nc = bacc.Bacc("TRN2", target_bir_lowering=False, debug=False, num_devices=8)
    xt = nc.dram_tensor("xt", [N, NH, KH, F], FP16, kind="ExternalInput").ap()
    at = nc.dram_tensor("at", [2, NH, NH], FP16, kind="ExternalInput").ap()
    sd = nc.dram_tensor("sd", [4, 128], FP16, kind="ExternalInput").ap()
    rd = nc.dram_tensor("rd", [4, 2048], FP16, kind="ExternalInput").ap()
    out = nc.dram_tensor("out", [NH, F], FP32, kind="ExternalOutput").ap()

    starts = []
    s = 0
    for ci in CIS:
        starts.append(s)
        s += ci
    assert s == NH

    with TileContext(nc) as tc:
        with (
            tc.tile_pool(name="const", bufs=1) as cpool,
            tc.tile_pool(name="xs", bufs=1) as xpool,
            tc.tile_pool(name="ps", bufs=1, space="PSUM") as pspool,
        ):
            at_sb = []
            for h in range(2):
                t = cpool.tile([128, NH], FP16, name=f"at{h}", tag=f"at{h}")
                nc.sync.dma_start(
                    out=t[:, :],
                    in_=bass.AP(at.tensor, h * NH * NH, [[NH, 128], [1, NH]]),
                )
                at_sb.append(t)
            s_sb = cpool.tile([4, 128], FP16, name="s_sb", tag="s_sb")
            nc.sync.dma_start(out=s_sb[:, :], in_=sd[:, :])
            r_sb = cpool.tile([4, 2048], FP16, name="r_sb", tag="r_sb")
            nc.sync.dma_start(out=r_sb[:, :], in_=rd[:, :])

            # X chunk DMAs, all issued up front so the HWDGE ring never
            # stalls behind a semaphore wait from the out-DMAs.
            xts = {}
            for c, (s0, ci) in enumerate(zip(starts, CIS)):
                for h in range(2):
                    t = xpool.tile(
                        [128, ci * CW], FP16, name=f"x{h}_{c}", tag=f"x{h}_{c}"
                    )
                    src = bass.AP(
                        xt.tensor,
                        (h * 128) * (NH * CW) + s0 * CW,
                        [[NH * CW, 128], [1, ci * CW]],
                    )
                    nc.sync.dma_start(out=t[:, :], in_=src)
                    xts[(h, c)] = t

            ps = [
                pspool.tile([128, 512], FP32, name=f"ps{b}", tag=f"ps{b}")
                for b in range(4)
            ]
            warm = pspool.tile([128, 512], FP32, name="warm", tag="warm")
            E = cpool.tile([128, 2048], FP32, name="E", tag="E")

            # HAM warmup: PE busy >=3.4us during the initial DMA stream.
            for w in range(NWARM):
                nc.tensor.matmul(
                    warm[:, 0:128], at_sb[0][:, :], at_sb[0][:, :],
                    start=True, stop=True,
                )

            e_step = E.ap[0][0]

            def chunk_of(i):
                for c in range(len(CIS)):
                    if i < starts[c] + CIS[c]:
                        return c, i - starts[c]
                raise AssertionError

            for b in range(4):
                # Seed bank b: one K=4 one-hot matmul writes d[i,:] into
                # row 32*(i%4), cols 64*((i%32)//4) for the bank's 32 i's,
                # zeros elsewhere, and sets every has_written bit.
                nc.tensor.matmul(
                    ps[b][:, :],
                    s_sb[0:4, :],
                    r_sb[0:4, b * 512:(b + 1) * 512],
                    start=True,
                    stop=False,
                    skip_group_check=True,
                )
                for g in range(8):          # column group (s = g)
                    for t in range(6):      # h = t // 3, k = t % 3
                        h, k = t // 3, t % 3
                        for m in range(4):  # strip = m, round-robin
                            q = 4 * g + m
                            i = 32 * b + q
                            c, ir = chunk_of(i)
                            nc.tensor.matmul(
                                ps[b][32 * m:32 * m + 1, g * F:(g + 1) * F],
                                at_sb[h][:, i:i + 1],
                                xts[(h, c)][
                                    :, (ir * KH + k) * F:(ir * KH + k + 1) * F
                                ],
                                start=False,
                                stop=(g == 7 and t == 5 and m == 3),
                                skip_group_check=True,
                                tile_position=(0, 32 * m),
                            )
                # Evacuate bank b (ScalarE reads PSUM fast), then one
                # strided DMA writes rows {0,32,64,96}x8 slots straight to
                # HBM rows 32b..32b+31 in natural i order (i = 32b+4s+m).
                nc.scalar.copy(E[:, b * 512:(b + 1) * 512], ps[b][:, :])
                src = bass.AP(
                    E.tensor,
                    E.offset + b * 512,
                    [[32 * e_step, 4], [F, 8], [1, F]],
                )
                dst = bass.AP(
                    out.tensor, 32 * b * F, [[F, 4], [4 * F, 8], [1, F]]
                )
                nc.sync.dma_start(out=dst, in_=src)

    nc.compile()
    _CACHE["nc"] = nc
    return nc


def _make_in_maps(A, X):
    idx = np.arange(NH)
    S = np.zeros((4, 128), dtype=np.float16)
    for q in range(4):
        S[q, 32 * q] = 1.0
    Xh = X[:, 1:4].astype(np.float16)  # (4, 3, 256, 256, 64)
    in_maps = []
    for c in range(8):
        b, h = c // 2, c % 2
        lo = h * NH
        # X_t[j, i, k, f] = X[b, k+1, lo+i, j, f]
        xtv = np.ascontiguousarray(Xh[b, :, lo:lo + NH].transpose(2, 1, 0, 3))
        atv = np.ascontiguousarray(
            A[b, lo:lo + NH, :].T.astype(np.float16)
        ).reshape(2, NH, NH)
        dv = X[b, 0, lo + idx, lo + idx, :].astype(np.float16)  # (128, 64)
        # R[m, b4*512 + s*64 + f] = d[32*b4 + 4*s + m, f]
        rv = np.ascontiguousarray(
            dv.reshape(4, 8, 4, F).transpose(2, 0, 1, 3)
        ).reshape(4, 2048)
        in_maps.append({"xt": xtv, "at": atv, "sd": S, "rd": rv})
    return in_maps


def run(A, X, trace=False, **kw):
    nc = _build_nc()
    in_maps = _make_in_maps(A, X)
    res = run_bass_kernel_spmd(
        nc, in_maps, core_ids=list(range(8)), trace=trace, **kw
    )
    out = np.empty((BATCH, N, F), dtype=np.float32)
    for c in range(8):
        b, h = c // 2, c % 2
        out[b, h * NH:(h + 1) * NH] = res.results[c]["out"]
    return out, res


def kernel(A, X):
    A = np.asarray(A, dtype=np.float32)
    X = np.asarray(X, dtype=np.float32)
    out, _ = run(A, X, trace=False)
    return out
